# revision 15
# baseline (speedup 1.0000x reference)
"""CPSpatialAttention Trainium2 kernel.

Sharding: data-parallel over B — each of the 8 NeuronCores processes one
batch image (M=1024 patches).  Two launches with a tiny host-side CP-ALS
between them (only cov_big.mean(0) couples the cores).

Phase 1 (per core):  per-patch gram -> Sc (softmax with a global exp
    shift), centering-matrix matmul + second gram -> cov_big, on-chip
    accumulation of sum_n cov[n].  Patch pairs run concurrently on the
    PE array via tile_position (even patch on rows/cols 0-63, odd patch
    on 64-127, fed by a patch-shifted copy of the row in partitions
    64-127).
Host:  cov_global reduction, exact jax CP-ALS replica -> cov_cp, and
    transposed scratch layouts for phase 2 (host transposes are cheap;
    on-device ones are not).
Phase 2 (per core):  EcT = FM^T @ (Sc^T + cov_cp^T) per patch, written
    in patch-major layout; out = x*(beta*Ec + x); host folds back.
"""

import sys

sys.path.insert(0, "/opt/trn_rl_repo")

import numpy as np

import concourse.bass as bass
import concourse.tile as tile
from concourse import mybir
from concourse.bass_utils import run_bass_kernel_spmd
from concourse.vector_clock import ScopedClock

# ---------------------------------------------------------------- constants
B, C, H, W = 8, 64, 224, 224
PH = PW = 7
HB, WB = H // PH, W // PW          # 32, 32
M = HB * WB                        # 1024 patches per image
P = PH * PW                        # 49
RANK, N_ITER = 8, 5
EXPK = 60.0                        # global exp shift for softmax stability
F32 = mybir.dt.float32
JB = 4                             # patch blocks per row
TB = WB // JB                      # 8 patches per block (4 pairs)
NU = TB // 2                       # pairs per block
RW = WB * P                        # row width in patch-major layout (1568)


# ------------------------------------------------------- walrus drain patch
# This container's walrus build rejects instructions carrying more than
# one semaphore wait ("Too many sync wait commands").  Two workarounds:
# the tail drain gets its waits hoisted onto single-wait nops, and a
# post-pass splits any remaining multi-wait/multi-update instruction.
def _patched_drain_and_barrier(self, tick_clock, wait_clock):
    nc = self.nc
    probe = nc.sync.nop(nofuse=True)
    wait_clock.add_sem_waits(probe.ins, ScopedClock({None: tick_clock.global_clock}))
    si = probe.ins.sync_info
    waits = list(si.on_wait) if si is not None else []
    if si is not None:
        si.on_wait = waits[:1]
    for w in waits[1:]:
        n = nc.sync.nop(nofuse=True)
        n.ins.sync_info = mybir.SyncInfo(on_wait=[w], on_update=[])
    nc.sync.drain()
    nc.all_engine_barrier()
    assert self.sems is not None
    popped = nc._tile_sem_poison_stack.pop()
    assert popped is self._sem_poison
    nc.clear_and_free_semaphores(list(self.sems.allocated().values()))
    nc.all_engine_barrier()


tile.TileContext._drain_and_barrier = _patched_drain_and_barrier


def _split_sync_waits(nc, limit=1):
    for f in nc.m.functions:
        for bb in f.blocks:
            new_list = []
            for ins in bb.instructions:
                si = ins.sync_info
                if si is not None and len(si.on_wait) > limit:
                    waits = list(si.on_wait)
                    si.on_wait = waits[:limit]
                    for w in waits[limit:]:
                        nop = mybir.InstNoOp(name=f"I-{nc.next_id()}", ins=[], outs=[])
                        nop.engine = ins.engine
                        nop.sync_info = mybir.SyncInfo(on_wait=[w], on_update=[])
                        new_list.append(nop)
                if si is not None and len(si.on_update) > limit:
                    assert not isinstance(ins, mybir.InstDMA), (
                        "cannot split updates on a DMA instruction"
                    )
                    updates = list(si.on_update)
                    si.on_update = updates[:limit]
                    new_list.append(ins)
                    for u in updates[limit:]:
                        nop = mybir.InstNoOp(name=f"I-{nc.next_id()}", ins=[], outs=[])
                        nop.engine = ins.engine
                        nop.sync_info = mybir.SyncInfo(on_wait=[], on_update=[u])
                        new_list.append(nop)
                    continue
                new_list.append(ins)
            bb.instructions[:] = new_list


def _mid_bcast(ap, count):
    """Insert a step-0 dim after the partition dim: [P, F...] -> [P, count, F...]."""
    return bass.AP(
        tensor=ap.tensor, offset=ap.offset, ap=[ap.ap[0], [0, count]] + list(ap.ap[1:])
    )


# ---------------------------------------------------------------- phase 1
def build_phase1():
    nc = bass.Bass()
    # x2: patch-major x  [C, HB, WB*P], x2[c, hb, wb*49 + ph*7 + pw]
    x2 = nc.declare_dram_parameter("x2", [C, HB, RW], F32, isOutput=False)
    cdiag = nc.declare_dram_parameter("cdiag", [128, 128], F32, isOutput=False)
    sc_o = nc.declare_dram_parameter("sc", [M, P, P], F32, isOutput=True)
    cov_o = nc.declare_dram_parameter("cov", [M, P, P], F32, isOutput=True)
    csum_o = nc.declare_dram_parameter("covsum", [2, P, P], F32, isOutput=True)

    with tile.TileContext(nc) as tc:
        with (
            tc.tile_pool(name="const", bufs=1) as const,
            tc.tile_pool(name="xrow", bufs=3) as xa,
            tc.tile_pool(name="work", bufs=3) as wk,
            tc.tile_pool(name="row", bufs=2) as rowp,
            tc.tile_pool(name="acc", bufs=1) as accp,
            tc.tile_pool(name="ps_s", bufs=3, space="PSUM") as ps_s,
            tc.tile_pool(name="ps_f", bufs=2, space="PSUM") as ps_f,
            tc.tile_pool(name="ps_c", bufs=3, space="PSUM") as ps_c,
        ):
            cdiag_s = const.tile([128, 128], F32)
            nc.sync.dma_start(out=cdiag_s[:], in_=cdiag[:])
            negk = const.tile([128, 1], F32)
            nc.vector.memset(negk[:], -EXPK)
            covacc = accp.tile([128, P], F32)
            nc.vector.memset(covacc[:], 0.0)

            for hb in range(HB):
                # top half: row patches; bottom half: shifted by one patch
                xf = xa.tile([128, RW], F32, tag="xf")
                nc.sync.dma_start(out=xf[0:C, :], in_=x2[:, hb, :])
                nc.sync.dma_start(out=xf[C:128, 0 : RW - P], in_=x2[:, hb, P:])
                sc_row = rowp.tile([128, WB // 2, P], F32, tag="scrow")
                cov_row = rowp.tile([128, WB // 2, P], F32, tag="covrow")
                for jb in range(JB):
                    n0 = hb * WB + jb * TB
                    u0 = jb * NU
                    # ---- centered FM^T: one matmul per block; top half
                    # centers patches jb*8..jb*8+7, bottom the same +1
                    # (only even columns of top / odd-covering columns of
                    # bottom are consumed below)
                    fmc_ps = ps_f.tile([128, TB, P], F32)
                    nc.tensor.matmul(
                        fmc_ps[:],
                        lhsT=cdiag_s[:],
                        rhs=xf[:, jb * TB * P : (jb + 1) * TB * P],
                        start=True,
                        stop=True,
                    )
                    fmc_s = wk.tile([128, TB, P], F32, tag="fmc")
                    nc.scalar.activation(
                        out=fmc_s[:],
                        in_=fmc_ps[:],
                        func=mybir.ActivationFunctionType.Copy,
                    )
                    # ---- gram matmuls: even patch rows 0-48, odd rows 64-112
                    s_ps = ps_s.tile([128, NU, P], F32)
                    for u in range(NU):
                        off = (jb * TB + 2 * u) * P
                        pe = xf[0:C, off : off + P]
                        po = xf[C:128, off : off + P]
                        nc.tensor.matmul(s_ps[0:P, u], lhsT=pe, rhs=pe, start=True, stop=True)
                        nc.tensor.matmul(
                            s_ps[C : C + P, u], lhsT=po, rhs=po, start=True, stop=True
                        )
                    # ---- softmax with global shift (both halves at once)
                    e_s = wk.tile([128, NU, P], F32, tag="es")
                    nc.scalar.activation(
                        out=e_s[:],
                        in_=s_ps[:],
                        func=mybir.ActivationFunctionType.Exp,
                        bias=negk[:],
                        scale=1.0,
                    )
                    rs = wk.tile([128, NU], F32, tag="rs")
                    nc.vector.reduce_sum(rs[:], e_s[:], axis=mybir.AxisListType.X)
                    rr = wk.tile([128, NU], F32, tag="rr")
                    nc.vector.reciprocal(rr[:], rs[:])
                    nc.vector.tensor_mul(
                        sc_row[:, u0 : u0 + NU],
                        e_s[:],
                        rr[:, :, None].to_broadcast((128, NU, P)),
                    )
                    # ---- covariance grams (centered), /64 on copy-out
                    cov_ps = ps_c.tile([128, NU, P], F32)
                    for u in range(NU):
                        fe = fmc_s[0:C, 2 * u]
                        fo = fmc_s[C:128, 2 * u]
                        nc.tensor.matmul(
                            cov_ps[0:P, u], lhsT=fe, rhs=fe, start=True, stop=True
                        )
                        nc.tensor.matmul(
                            cov_ps[C : C + P, u], lhsT=fo, rhs=fo, start=True, stop=True
                        )
                    nc.scalar.activation(
                        out=cov_row[:, u0 : u0 + NU],
                        in_=cov_ps[:],
                        func=mybir.ActivationFunctionType.Copy,
                        scale=1.0 / C,
                    )
                    # ---- accumulate sum over patches of cov (both halves)
                    part = wk.tile([128, P], F32, tag="part")
                    nc.vector.reduce_sum(
                        part[:],
                        cov_ps.rearrange("p t q -> p q t"),
                        axis=mybir.AxisListType.X,
                    )
                    nc.vector.tensor_add(covacc[0:P], covacc[0:P], part[0:P])
                    nc.vector.tensor_add(
                        covacc[C : C + P], covacc[C : C + P], part[C : C + P]
                    )
                # ---- row stores (even/odd interleave via strided n)
                n0 = hb * WB
                nc.gpsimd.dma_start(
                    out=sc_o[n0 : n0 + WB : 2].rearrange("n p q -> p n q"),
                    in_=sc_row[0:P],
                )
                nc.gpsimd.dma_start(
                    out=sc_o[n0 + 1 : n0 + WB : 2].rearrange("n p q -> p n q"),
                    in_=sc_row[C : C + P],
                )
                nc.gpsimd.dma_start(
                    out=cov_o[n0 : n0 + WB : 2].rearrange("n p q -> p n q"),
                    in_=cov_row[0:P],
                )
                nc.gpsimd.dma_start(
                    out=cov_o[n0 + 1 : n0 + WB : 2].rearrange("n p q -> p n q"),
                    in_=cov_row[C : C + P],
                )

            nc.sync.dma_start(out=csum_o[0], in_=covacc[0:P])
            nc.sync.dma_start(out=csum_o[1], in_=covacc[C : C + P])
    _split_sync_waits(nc)
    return nc


# ---------------------------------------------------------------- phase 2
def build_phase2():
    nc = bass.Bass()
    s2 = nc.declare_dram_parameter("s2", [P, M, P], F32, isOutput=False)
    f2 = nc.declare_dram_parameter("f2", [P, M, C], F32, isOutput=False)
    covt = nc.declare_dram_parameter("covt", [P, P], F32, isOutput=False)
    ec_o = nc.declare_dram_parameter("ec2", [C, HB, RW], F32, isOutput=True)

    with tile.TileContext(nc) as tc:
        with (
            tc.tile_pool(name="const", bufs=1) as const,
            tc.tile_pool(name="big", bufs=3) as big,
            tc.tile_pool(name="ps_e", bufs=4, space="PSUM") as ps_e,
        ):
            covt_s = const.tile([P, P], F32)
            nc.sync.dma_start(out=covt_s[:], in_=covt[:])

            for hb in range(HB):
                n0 = hb * WB
                # L^T tiles for the whole row: Sc^T + cov_cp^T
                lt = big.tile([P, WB, P], F32, tag="lt")
                nc.sync.dma_start(out=lt[:], in_=s2[:, n0 : n0 + WB, :])
                nc.vector.tensor_add(lt[:], lt[:], _mid_bcast(covt_s[:], WB))
                # FM tiles [P(q), WB, C]
                fm = big.tile([P, WB, C], F32, tag="fm")
                nc.sync.dma_start(out=fm[:], in_=f2[:, n0 : n0 + WB, :])

                ec_row = big.tile([C, RW], F32, tag="ecrow")
                for jb in range(JB):
                    ec_ps = ps_e.tile([C, TB, P], F32)
                    for t in range(TB):
                        wb = jb * TB + t
                        nc.tensor.matmul(
                            ec_ps[:, t],
                            lhsT=fm[:, wb],
                            rhs=lt[:, wb],
                            start=True,
                            stop=True,
                        )
                    nc.scalar.activation(
                        out=ec_row[:, jb * TB * P : (jb + 1) * TB * P],
                        in_=ec_ps[:],
                        func=mybir.ActivationFunctionType.Copy,
                    )
                nc.sync.dma_start(out=ec_o[:, hb, :], in_=ec_row[:])
    _split_sync_waits(nc)
    return nc


# ---------------------------------------------------------------- host CP-ALS
def _khatri_rao(a, b):
    return (a[:, None, :] * b[None, :, :]).reshape(-1, a.shape[1])


def _cp_cov(cov_global):
    """Exact replica of reference cp_als on the [1,P,P] global covariance,
    returning cov_cp = (f1 * f0[0]) @ f2.T  (weights are ones)."""
    import jax
    import jax.numpy as jnp

    cpu = jax.devices("cpu")[0]
    with jax.default_device(cpu):
        t = jnp.asarray(cov_global, dtype=jnp.float32)
        dims = t.shape
        key = jax.random.key(42)
        keys = jax.random.split(key, len(dims))
        factors = [
            jax.random.uniform(keys[i], (dims[i], RANK), dtype=t.dtype)
            for i in range(len(dims))
        ]
        unfoldings = [
            jnp.moveaxis(t, n, 0).reshape(t.shape[n], -1) for n in range(len(dims))
        ]
        for _ in range(N_ITER):
            for n in range(len(dims)):
                others = [factors[m] for m in range(len(dims)) if m != n]
                kr = _khatri_rao(others[0], others[1])
                mttkrp = unfoldings[n] @ kr
                v = (others[0].T @ others[0]) * (others[1].T @ others[1])
                factors[n] = mttkrp @ jnp.linalg.pinv(v)
        f0, f1, f2 = factors
        w = f0[0]
        cov_cp = (f1 * w[None, :]) @ f2.T
        return np.asarray(cov_cp, dtype=np.float32)


# ---------------------------------------------------------------- driver
_NC_CACHE = {}
LAST_STATS = {}


def _get_nc(name, builder):
    if name not in _NC_CACHE:
        _NC_CACHE[name] = builder()
    return _NC_CACHE[name]


def kernel(x, beta):
    x = np.ascontiguousarray(np.asarray(x, dtype=np.float32))
    beta = np.asarray(beta, dtype=np.float32)
    cores = list(range(B))

    # patch-major x: [B, C, HB, WB, PH, PW] -> [B, C, HB, WB*P]
    x2 = np.ascontiguousarray(
        x.reshape(B, C, HB, PH, WB, PW).transpose(0, 1, 2, 4, 3, 5)
    ).reshape(B, C, HB, RW)

    # ---- phase 1
    cmat = (np.eye(C, dtype=np.float32) - np.float32(1.0 / C)).astype(np.float32)
    cdiag = np.zeros((128, 128), dtype=np.float32)
    cdiag[:C, :C] = cmat
    cdiag[C:, C:] = cmat
    nc1 = _get_nc("p1", build_phase1)
    in1 = [{"x2": x2[b], "cdiag": cdiag} for b in range(B)]
    rr1 = run_bass_kernel_spmd(nc1, in1, cores)
    LAST_STATS["p1_exec_ns"] = rr1.exec_time_ns
    r1 = rr1.results

    sc = np.stack([r1[b]["sc"] for b in range(B)])        # [B, M, P, P]
    cov = np.stack([r1[b]["cov"] for b in range(B)])      # [B, M, P, P]
    covsum = np.stack([r1[b]["covsum"] for b in range(B)])

    # ---- host: global covariance + CP-ALS
    # covacc accumulated the raw (un-divided) gram PSUM values: /C here too
    cov_global = (covsum.sum(axis=(0, 1)) / np.float32(B * M * C))[None]  # [1, P, P]
    cov_cp = _cp_cov(cov_global)                                       # [P, P]

    # ---- scratch layouts for phase 2
    fmx = x.reshape(B, C, HB, PH, WB, PW).transpose(0, 2, 4, 3, 5, 1)
    fmx = np.ascontiguousarray(fmx).reshape(B, M, P, C)
    f2 = np.ascontiguousarray(fmx.transpose(0, 2, 1, 3))          # [B, P, M, C]
    s2 = np.ascontiguousarray(sc.transpose(0, 3, 1, 2))           # [B, P(q), M, P(p)]
    covt = np.ascontiguousarray(cov_cp.T)

    # ---- phase 2
    nc2 = _get_nc("p2", build_phase2)
    in2 = [{"s2": s2[b], "f2": f2[b], "covt": covt} for b in range(B)]
    rr2 = run_bass_kernel_spmd(nc2, in2, cores)
    LAST_STATS["p2_exec_ns"] = rr2.exec_time_ns
    r2 = rr2.results
    ec2 = np.stack([r2[b]["ec2"] for b in range(B)])

    def unfold(y2):
        y = y2.reshape(B, C, HB, WB, PH, PW).transpose(0, 1, 2, 4, 3, 5)
        return np.ascontiguousarray(y).reshape(B, C, H, W)

    ec = unfold(ec2)
    out = x * (beta.astype(np.float32) * ec + x)
    return (out, sc, cov, ec)


# revision 16
# speedup vs baseline: 1.2098x; 1.2098x over previous
"""CPSpatialAttention Trainium2 kernel.

Sharding: data-parallel over B — each of the 8 NeuronCores processes one
batch image (M=1024 patches).  Two launches with a tiny host-side CP-ALS
between them (only cov_big.mean(0) couples the cores).

Phase 1 (per core):  per-patch gram -> Sc (softmax with a global exp
    shift), centering-matrix matmul + second gram -> cov_big, on-chip
    accumulation of sum_n cov[n].  Patch pairs run concurrently on the
    PE array via tile_position (even patch on rows/cols 0-63, odd patch
    on 64-127, fed by a patch-shifted copy of the row in partitions
    64-127).
Host:  cov_global reduction, exact jax CP-ALS replica -> cov_cp, and
    transposed scratch layouts for phase 2 (host transposes are cheap;
    on-device ones are not).
Phase 2 (per core):  EcT = FM^T @ (Sc^T + cov_cp^T) per patch, written
    in patch-major layout; out = x*(beta*Ec + x); host folds back.
"""

import sys

sys.path.insert(0, "/opt/trn_rl_repo")

import numpy as np

import concourse.bass as bass
import concourse.tile as tile
from concourse import mybir
from concourse.bass_utils import run_bass_kernel_spmd
from concourse.vector_clock import ScopedClock

# ---------------------------------------------------------------- constants
B, C, H, W = 8, 64, 224, 224
PH = PW = 7
HB, WB = H // PH, W // PW          # 32, 32
M = HB * WB                        # 1024 patches per image
P = PH * PW                        # 49
RANK, N_ITER = 8, 5
EXPK = 60.0                        # global exp shift for softmax stability
F32 = mybir.dt.float32
JB = 4                             # patch blocks per row
TB = WB // JB                      # 8 patches per block (4 pairs)
NU = TB // 2                       # pairs per block
RW = WB * P                        # row width in patch-major layout (1568)


# ------------------------------------------------------- walrus drain patch
# This container's walrus build rejects instructions carrying more than
# one semaphore wait ("Too many sync wait commands").  Two workarounds:
# the tail drain gets its waits hoisted onto single-wait nops, and a
# post-pass splits any remaining multi-wait/multi-update instruction.
def _patched_drain_and_barrier(self, tick_clock, wait_clock):
    nc = self.nc
    probe = nc.sync.nop(nofuse=True)
    wait_clock.add_sem_waits(probe.ins, ScopedClock({None: tick_clock.global_clock}))
    si = probe.ins.sync_info
    waits = list(si.on_wait) if si is not None else []
    if si is not None:
        si.on_wait = waits[:1]
    for w in waits[1:]:
        n = nc.sync.nop(nofuse=True)
        n.ins.sync_info = mybir.SyncInfo(on_wait=[w], on_update=[])
    nc.sync.drain()
    nc.all_engine_barrier()
    assert self.sems is not None
    popped = nc._tile_sem_poison_stack.pop()
    assert popped is self._sem_poison
    nc.clear_and_free_semaphores(list(self.sems.allocated().values()))
    nc.all_engine_barrier()


tile.TileContext._drain_and_barrier = _patched_drain_and_barrier


def _split_sync_waits(nc, limit=1):
    for f in nc.m.functions:
        for bb in f.blocks:
            new_list = []
            for ins in bb.instructions:
                si = ins.sync_info
                if si is not None and len(si.on_wait) > limit:
                    waits = list(si.on_wait)
                    si.on_wait = waits[:limit]
                    for w in waits[limit:]:
                        nop = mybir.InstNoOp(name=f"I-{nc.next_id()}", ins=[], outs=[])
                        nop.engine = ins.engine
                        nop.sync_info = mybir.SyncInfo(on_wait=[w], on_update=[])
                        new_list.append(nop)
                if si is not None and len(si.on_update) > limit:
                    assert not isinstance(ins, mybir.InstDMA), (
                        "cannot split updates on a DMA instruction"
                    )
                    updates = list(si.on_update)
                    si.on_update = updates[:limit]
                    new_list.append(ins)
                    for u in updates[limit:]:
                        nop = mybir.InstNoOp(name=f"I-{nc.next_id()}", ins=[], outs=[])
                        nop.engine = ins.engine
                        nop.sync_info = mybir.SyncInfo(on_wait=[], on_update=[u])
                        new_list.append(nop)
                    continue
                new_list.append(ins)
            bb.instructions[:] = new_list


def _mid_bcast(ap, count):
    """Insert a step-0 dim after the partition dim: [P, F...] -> [P, count, F...]."""
    return bass.AP(
        tensor=ap.tensor, offset=ap.offset, ap=[ap.ap[0], [0, count]] + list(ap.ap[1:])
    )


# ---------------------------------------------------------------- phase 1
def build_phase1():
    nc = bass.Bass()
    # x2: patch-major x  [C, HB, WB*P], x2[c, hb, wb*49 + ph*7 + pw]
    x2 = nc.declare_dram_parameter("x2", [C, HB, RW], F32, isOutput=False)
    cdiag = nc.declare_dram_parameter("cdiag", [128, 128], F32, isOutput=False)
    sc_o = nc.declare_dram_parameter("sc", [M, P, P], F32, isOutput=True)
    cov_o = nc.declare_dram_parameter("cov", [M, P, P], F32, isOutput=True)
    csum_o = nc.declare_dram_parameter("covsum", [2, P, P], F32, isOutput=True)

    with tile.TileContext(nc) as tc:
        with (
            tc.tile_pool(name="const", bufs=1) as const,
            tc.tile_pool(name="xrow", bufs=4) as xa,
            tc.tile_pool(name="work", bufs=4) as wk,
            tc.tile_pool(name="row", bufs=3) as rowp,
            tc.tile_pool(name="acc", bufs=1) as accp,
            tc.tile_pool(name="ps_s", bufs=3, space="PSUM") as ps_s,
            tc.tile_pool(name="ps_f", bufs=2, space="PSUM") as ps_f,
            tc.tile_pool(name="ps_c", bufs=3, space="PSUM") as ps_c,
        ):
            cdiag_s = const.tile([128, 128], F32)
            nc.sync.dma_start(out=cdiag_s[:], in_=cdiag[:])
            negk = const.tile([128, 1], F32)
            nc.vector.memset(negk[:], -EXPK)
            covacc = accp.tile([128, P], F32)
            nc.vector.memset(covacc[:], 0.0)

            for hb in range(HB):
                # top half: row patches; bottom half: shifted by one patch
                xf = xa.tile([128, RW], F32, tag="xf")
                nc.sync.dma_start(out=xf[0:C, :], in_=x2[:, hb, :])
                nc.sync.dma_start(out=xf[C:128, 0 : RW - P], in_=x2[:, hb, P:])
                sc_row = rowp.tile([128, WB // 2, P], F32, tag="scrow")
                cov_row = rowp.tile([128, WB // 2, P], F32, tag="covrow")
                for jb in range(JB):
                    n0 = hb * WB + jb * TB
                    u0 = jb * NU
                    # ---- centered FM^T: one matmul per block; top half
                    # centers patches jb*8..jb*8+7, bottom the same +1
                    # (only even columns of top / odd-covering columns of
                    # bottom are consumed below)
                    fmc_ps = ps_f.tile([128, TB, P], F32)
                    nc.tensor.matmul(
                        fmc_ps[:],
                        lhsT=cdiag_s[:],
                        rhs=xf[:, jb * TB * P : (jb + 1) * TB * P],
                        start=True,
                        stop=True,
                    )
                    fmc_s = wk.tile([128, TB, P], F32, tag="fmc")
                    nc.scalar.activation(
                        out=fmc_s[:],
                        in_=fmc_ps[:],
                        func=mybir.ActivationFunctionType.Copy,
                    )
                    # ---- gram matmuls: even patch rows 0-48, odd rows 64-112
                    s_ps = ps_s.tile([128, NU, P], F32)
                    for u in range(NU):
                        off = (jb * TB + 2 * u) * P
                        pe = xf[0:C, off : off + P]
                        po = xf[C:128, off : off + P]
                        nc.tensor.matmul(s_ps[0:P, u], lhsT=pe, rhs=pe, start=True, stop=True)
                        nc.tensor.matmul(
                            s_ps[C : C + P, u], lhsT=po, rhs=po, start=True, stop=True
                        )
                    # ---- softmax with global shift (both halves at once)
                    e_s = wk.tile([128, NU, P], F32, tag="es")
                    nc.scalar.activation(
                        out=e_s[:],
                        in_=s_ps[:],
                        func=mybir.ActivationFunctionType.Exp,
                        bias=negk[:],
                        scale=1.0,
                    )
                    rs = wk.tile([128, NU], F32, tag="rs")
                    nc.vector.reduce_sum(rs[:], e_s[:], axis=mybir.AxisListType.X)
                    rr = wk.tile([128, NU], F32, tag="rr")
                    nc.vector.reciprocal(rr[:], rs[:])
                    nc.vector.tensor_mul(
                        sc_row[:, u0 : u0 + NU],
                        e_s[:],
                        rr[:, :, None].to_broadcast((128, NU, P)),
                    )
                    # ---- covariance grams (centered), /64 on copy-out
                    cov_ps = ps_c.tile([128, NU, P], F32)
                    for u in range(NU):
                        fe = fmc_s[0:C, 2 * u]
                        fo = fmc_s[C:128, 2 * u]
                        nc.tensor.matmul(
                            cov_ps[0:P, u], lhsT=fe, rhs=fe, start=True, stop=True
                        )
                        nc.tensor.matmul(
                            cov_ps[C : C + P, u], lhsT=fo, rhs=fo, start=True, stop=True
                        )
                    nc.scalar.activation(
                        out=cov_row[:, u0 : u0 + NU],
                        in_=cov_ps[:],
                        func=mybir.ActivationFunctionType.Copy,
                        scale=1.0 / C,
                    )
                # ---- accumulate sum over patches of cov (per row, from SBUF)
                part = wk.tile([128, P], F32, tag="part")
                nc.vector.reduce_sum(
                    part[:],
                    cov_row.rearrange("p t q -> p q t"),
                    axis=mybir.AxisListType.X,
                )
                nc.vector.tensor_add(covacc[0:P], covacc[0:P], part[0:P])
                nc.vector.tensor_add(
                    covacc[C : C + P], covacc[C : C + P], part[C : C + P]
                )
                # ---- row stores (even/odd interleave via strided n)
                n0 = hb * WB
                nc.gpsimd.dma_start(
                    out=sc_o[n0 : n0 + WB : 2].rearrange("n p q -> p n q"),
                    in_=sc_row[0:P],
                )
                nc.gpsimd.dma_start(
                    out=sc_o[n0 + 1 : n0 + WB : 2].rearrange("n p q -> p n q"),
                    in_=sc_row[C : C + P],
                )
                nc.gpsimd.dma_start(
                    out=cov_o[n0 : n0 + WB : 2].rearrange("n p q -> p n q"),
                    in_=cov_row[0:P],
                )
                nc.gpsimd.dma_start(
                    out=cov_o[n0 + 1 : n0 + WB : 2].rearrange("n p q -> p n q"),
                    in_=cov_row[C : C + P],
                )

            nc.sync.dma_start(out=csum_o[0], in_=covacc[0:P])
            nc.sync.dma_start(out=csum_o[1], in_=covacc[C : C + P])
    _split_sync_waits(nc)
    return nc


# ---------------------------------------------------------------- phase 2
def build_phase2():
    nc = bass.Bass()
    s2 = nc.declare_dram_parameter("s2", [P, M, P], F32, isOutput=False)
    f2 = nc.declare_dram_parameter("f2", [P, M, C], F32, isOutput=False)
    covt = nc.declare_dram_parameter("covt", [P, P], F32, isOutput=False)
    ec_o = nc.declare_dram_parameter("ec2", [C, HB, RW], F32, isOutput=True)

    with tile.TileContext(nc) as tc:
        with (
            tc.tile_pool(name="const", bufs=1) as const,
            tc.tile_pool(name="big", bufs=3) as big,
            tc.tile_pool(name="ps_e", bufs=4, space="PSUM") as ps_e,
        ):
            covt_s = const.tile([P, P], F32)
            nc.sync.dma_start(out=covt_s[:], in_=covt[:])

            for hb in range(HB):
                n0 = hb * WB
                # L^T tiles for the whole row: Sc^T + cov_cp^T
                lt = big.tile([P, WB, P], F32, tag="lt")
                nc.sync.dma_start(out=lt[:], in_=s2[:, n0 : n0 + WB, :])
                nc.vector.tensor_add(lt[:], lt[:], _mid_bcast(covt_s[:], WB))
                # FM tiles [P(q), WB, C]
                fm = big.tile([P, WB, C], F32, tag="fm")
                nc.sync.dma_start(out=fm[:], in_=f2[:, n0 : n0 + WB, :])

                ec_row = big.tile([C, RW], F32, tag="ecrow")
                for jb in range(JB):
                    ec_ps = ps_e.tile([C, TB, P], F32)
                    for t in range(TB):
                        wb = jb * TB + t
                        nc.tensor.matmul(
                            ec_ps[:, t],
                            lhsT=fm[:, wb],
                            rhs=lt[:, wb],
                            start=True,
                            stop=True,
                        )
                    nc.scalar.activation(
                        out=ec_row[:, jb * TB * P : (jb + 1) * TB * P],
                        in_=ec_ps[:],
                        func=mybir.ActivationFunctionType.Copy,
                    )
                nc.sync.dma_start(out=ec_o[:, hb, :], in_=ec_row[:])
    _split_sync_waits(nc)
    return nc


# ---------------------------------------------------------------- host CP-ALS
def _khatri_rao(a, b):
    return (a[:, None, :] * b[None, :, :]).reshape(-1, a.shape[1])


def _cp_cov(cov_global):
    """Exact replica of reference cp_als on the [1,P,P] global covariance,
    returning cov_cp = (f1 * f0[0]) @ f2.T  (weights are ones)."""
    import jax
    import jax.numpy as jnp

    cpu = jax.devices("cpu")[0]
    with jax.default_device(cpu):
        t = jnp.asarray(cov_global, dtype=jnp.float32)
        dims = t.shape
        key = jax.random.key(42)
        keys = jax.random.split(key, len(dims))
        factors = [
            jax.random.uniform(keys[i], (dims[i], RANK), dtype=t.dtype)
            for i in range(len(dims))
        ]
        unfoldings = [
            jnp.moveaxis(t, n, 0).reshape(t.shape[n], -1) for n in range(len(dims))
        ]
        for _ in range(N_ITER):
            for n in range(len(dims)):
                others = [factors[m] for m in range(len(dims)) if m != n]
                kr = _khatri_rao(others[0], others[1])
                mttkrp = unfoldings[n] @ kr
                v = (others[0].T @ others[0]) * (others[1].T @ others[1])
                factors[n] = mttkrp @ jnp.linalg.pinv(v)
        f0, f1, f2 = factors
        w = f0[0]
        cov_cp = (f1 * w[None, :]) @ f2.T
        return np.asarray(cov_cp, dtype=np.float32)


# ---------------------------------------------------------------- driver
_NC_CACHE = {}
LAST_STATS = {}


def _get_nc(name, builder):
    if name not in _NC_CACHE:
        _NC_CACHE[name] = builder()
    return _NC_CACHE[name]


def kernel(x, beta):
    x = np.ascontiguousarray(np.asarray(x, dtype=np.float32))
    beta = np.asarray(beta, dtype=np.float32)
    cores = list(range(B))

    # patch-major x: [B, C, HB, WB, PH, PW] -> [B, C, HB, WB*P]
    x2 = np.ascontiguousarray(
        x.reshape(B, C, HB, PH, WB, PW).transpose(0, 1, 2, 4, 3, 5)
    ).reshape(B, C, HB, RW)

    # ---- phase 1
    cmat = (np.eye(C, dtype=np.float32) - np.float32(1.0 / C)).astype(np.float32)
    cdiag = np.zeros((128, 128), dtype=np.float32)
    cdiag[:C, :C] = cmat
    cdiag[C:, C:] = cmat
    nc1 = _get_nc("p1", build_phase1)
    in1 = [{"x2": x2[b], "cdiag": cdiag} for b in range(B)]
    rr1 = run_bass_kernel_spmd(nc1, in1, cores)
    LAST_STATS["p1_exec_ns"] = rr1.exec_time_ns
    r1 = rr1.results

    sc = np.stack([r1[b]["sc"] for b in range(B)])        # [B, M, P, P]
    cov = np.stack([r1[b]["cov"] for b in range(B)])      # [B, M, P, P]
    covsum = np.stack([r1[b]["covsum"] for b in range(B)])

    # ---- host: global covariance + CP-ALS
    cov_global = (covsum.sum(axis=(0, 1)) / np.float32(B * M))[None]  # [1, P, P]
    cov_cp = _cp_cov(cov_global)                                       # [P, P]

    # ---- scratch layouts for phase 2
    fmx = x.reshape(B, C, HB, PH, WB, PW).transpose(0, 2, 4, 3, 5, 1)
    fmx = np.ascontiguousarray(fmx).reshape(B, M, P, C)
    f2 = np.ascontiguousarray(fmx.transpose(0, 2, 1, 3))          # [B, P, M, C]
    s2 = np.ascontiguousarray(sc.transpose(0, 3, 1, 2))           # [B, P(q), M, P(p)]
    covt = np.ascontiguousarray(cov_cp.T)

    # ---- phase 2
    nc2 = _get_nc("p2", build_phase2)
    in2 = [{"s2": s2[b], "f2": f2[b], "covt": covt} for b in range(B)]
    rr2 = run_bass_kernel_spmd(nc2, in2, cores)
    LAST_STATS["p2_exec_ns"] = rr2.exec_time_ns
    r2 = rr2.results
    ec2 = np.stack([r2[b]["ec2"] for b in range(B)])

    def unfold(y2):
        y = y2.reshape(B, C, HB, WB, PH, PW).transpose(0, 1, 2, 4, 3, 5)
        return np.ascontiguousarray(y).reshape(B, C, H, W)

    ec = unfold(ec2)
    out = x * (beta.astype(np.float32) * ec + x)
    return (out, sc, cov, ec)
